# revision 1
# baseline (speedup 1.0000x reference)
"""Trainium2 Bass kernel for nn_CTRule (temporal KG scoring model).

Computes, for each of B=1024 queries (h, r, t):
  v = f(E0[h], E1[r], time tables, rule tables)   # [B, 128] elementwise algebra
  scores = v @ E0.T                               # [B, 40000]

Distribution over the 8 NeuronCores (pair-hybrid): the two cores of pair p
process batch tiles 2p, 2p+1 against disjoint halves of the 40000-entity
axis.  Per core: gather (indirect DMA) the per-example table rows, run the
elementwise head in fp16 on VectorE (+GpSimd for the independent rule
chain), transpose v on TensorE, stream this core's E0T half through
40-chunk matmuls per tile, and write the fp16 [256, 20000] block to HBM.

Latency structure (from trace analysis of prior versions):
  * idx is DMA'd by gpsimd itself (lands ~3us; the HWDGE rings only reach
    their first issue slot at ~5-7us after library loads).
  * gathers issue back-to-back on gpsimd right after idx; tile 0's tables
    first.  Tables are host-augmented with swapped halves ([x0|x1]->[x1|x0])
    so every complex-product pair is ONE wide [P,256] fp16 multiply.
  * the head is a single-engine chain on Vector (no cross-engine ping-pong)
    except the rule cmul which GpSimd computes concurrently.
  * E0T chunk loads run on the Activation HWDGE ring from ~7us (no deps).
  * matmul chunks are 512 cols (one PSUM bank); pairs share a [P,1024] PSUM
    tile drained by one copy (Vector/Scalar alternate); every 1024-col group
    is DMA'd to HBM on the Sync ring as soon as its copy lands, so the
    ~31us write stream overlaps everything else.
All head math in fp16 (rel err ~6e-4 total vs the 2e-2 gate).  No
cross-core communication; the host reassembles the 8 blocks.
"""

import numpy as np

P = 128
B = 1024
RANK = 128
NENT = 40000
NREL = 230
NTIME = 365
CYCLE = 120
NCORES = 8
NHALF = NENT // 2        # entity columns per core = 20000
CHUNK = 512              # matmul chunk columns (= one PSUM bank of f32)
LOADCH = 2500            # E0T load-chunk columns (8 loads of 0.64MB)
OUTCH = 1024             # output DMA group columns (= one copy group)

RC_W = 6 * RANK          # [E1 | E1sw | rule_C | rule_Csw | -rS*E1 | hrw] = 768
TC_W = 5 * RANK          # [E4 | E4 | TM | TE | TEsw] = 640
LH_W = 2 * RANK          # [E0row | E0row-swapped] = 256

TRACE = False            # set by test harness for profiling runs
_CACHE = {}


def _build():
    import concourse.bass as bass
    import concourse.mybir as mybir
    import concourse.tile as tile
    from concourse import bacc
    from concourse.masks import make_identity

    dt = mybir.dt
    mult = mybir.AluOpType.mult
    add = mybir.AluOpType.add
    sub = mybir.AluOpType.subtract

    nc = bacc.Bacc("TRN2", target_bir_lowering=False, debug=False,
                   num_devices=NCORES)

    IDX = nc.dram_tensor("IDX", [P, 8], dt.int32, kind="ExternalInput").ap()
    E0G = nc.dram_tensor("E0G", [NENT, LH_W], dt.float16, kind="ExternalInput").ap()
    RCAT = nc.dram_tensor("RCAT", [NREL, RC_W], dt.float16, kind="ExternalInput").ap()
    TCAT = nc.dram_tensor("TCAT", [NTIME, TC_W], dt.float16, kind="ExternalInput").ap()
    E0T = nc.dram_tensor("E0T", [RANK, NHALF], dt.float16, kind="ExternalInput").ap()
    OUT = nc.dram_tensor("OUT", [2 * P, NHALF], dt.float16, kind="ExternalOutput").ap()

    with tile.TileContext(nc) as tc:
        with (
            tc.tile_pool(name="const", bufs=1) as constp,
            tc.tile_pool(name="gath", bufs=1) as gp,
            tc.tile_pool(name="ew", bufs=1) as ew,
            tc.tile_pool(name="pst", bufs=1, space="PSUM") as pst,
            tc.tile_pool(name="psm", bufs=3, space="PSUM") as psm,
        ):
            # ---- idx as the sync ring's first issue (sync has no
            # act-table preamble, so its first DMA slot is earliest).
            idxt = gp.tile([P, 8], dt.int32)
            nc.sync.dma_start(idxt[:], IDX[:])

            # idx layout: cols (r0,r1, t0,t1, h0,h1, pad,pad); tile0 first
            lhsv = gp.tile([P, 2, LH_W], dt.float16, name="lhs")
            r8v = gp.tile([P, 2, RC_W], dt.float16, name="r8")
            t8v = gp.tile([P, 2, TC_W], dt.float16, name="t8")
            for j in range(2):
                for dst, src, col in ((r8v, RCAT, 0), (t8v, TCAT, 2),
                                      (lhsv, E0G, 4)):
                    nc.gpsimd.indirect_dma_start(
                        out=dst[:, j, :], out_offset=None, in_=src[:],
                        in_offset=bass.IndirectOffsetOnAxis(
                            ap=idxt[:, col + j:col + j + 1], axis=0))

            # ---- E0T half-table stream on the Activation HWDGE ring
            e0t = constp.tile([RANK, NHALF], dt.float16)
            for c0 in range(0, NHALF, LOADCH):
                nc.scalar.dma_start(e0t[:, c0:c0 + LOADCH],
                                    E0T[:, c0:c0 + LOADCH])

            ident = constp.tile([P, P], dt.float16)
            make_identity(nc, ident[:])

            def VTT(out, a, b_, op):
                nc.vector.tensor_tensor(out=out, in0=a, in1=b_, op=op)

            def GTT(out, a, b_, op):
                nc.gpsimd.tensor_tensor(out=out, in0=a, in1=b_, op=op)

            A = [ew.tile([P, RANK], dt.float16, name=f"A{j}") for j in range(2)]
            Bt = [ew.tile([P, RANK], dt.float16, name=f"B{j}") for j in range(2)]
            PA = [ew.tile([P, 2 * RANK], dt.float16, name=f"PA{j}") for j in range(2)]
            PB = [ew.tile([P, 2 * RANK], dt.float16, name=f"PB{j}") for j in range(2)]
            QQ = [ew.tile([P, 2 * RANK], dt.float16, name=f"QQ{j}") for j in range(2)]
            SS = [ew.tile([P, 2 * RANK], dt.float16, name=f"SS{j}") for j in range(2)]
            DD = [ew.tile([P, 2 * RANK], dt.float16, name=f"DD{j}") for j in range(2)]
            PL = [ew.tile([P, 2 * RANK], dt.float16, name=f"PL{j}") for j in range(2)]
            PT = [ew.tile([P, 2 * RANK], dt.float16, name=f"PT{j}") for j in range(2)]
            t0 = [ew.tile([P, 64], dt.float16, name=f"t0_{j}") for j in range(2)]
            t1 = [ew.tile([P, 64], dt.float16, name=f"t1_{j}") for j in range(2)]
            V = [ew.tile([P, RANK], dt.float16, name=f"V{j}") for j in range(2)]

            def head_rule(j, TT):
                # A = cmul(CT, RC) - rule_S*rel  (host table NRSREL = -rS*E1)
                # PA = [CT|CT]*[RC|RCsw] = [CT0RC0|CT1RC1 | CT0RC1|CT1RC0]
                r8 = r8v[:, j, :]
                TT(PA[j][:], t8v[:, j, 0:256], r8[:, 256:512], mult)
                TT(A[j][:, 0:64], PA[j][:, 0:64], PA[j][:, 64:128], sub)
                TT(A[j][:, 64:128], PA[j][:, 128:192], PA[j][:, 192:256], add)
                TT(A[j][:], A[j][:], r8[:, 512:640], add)

            def head_main(j, TT, full=True):
                r8 = r8v[:, j, :]
                t8 = t8v[:, j, :]
                lhs = lhsv[:, j, :]
                RELRELSW = r8[:, 0:256]
                REL = r8[:, 0:128]
                HRW = r8[:, 640:768]
                CT = t8[:, 0:128]
                TM = t8[:, 256:384]
                TM0 = t8[:, 256:320]
                TM1 = t8[:, 320:384]
                TESW2 = t8[:, 384:640]
                LHS = lhs[:, 0:128]
                Aj, Bj = A[j], Bt[j]
                # B = lhs + cmul(REL, LHS):
                # PB = [REL|RELsw]*[L|L] = [RL0L0|RL1L1 | RL1L0|RL0L1]
                TT(PB[j][:], RELRELSW, lhs[:, 0:256], mult)
                TT(Bj[:, 0:64], PB[j][:, 0:64], PB[j][:, 64:128], sub)
                TT(Bj[:, 64:128], PB[j][:, 128:192], PB[j][:, 192:256], add)
                TT(Bj[:], Bj[:], LHS, add)
                # A = rule_score = B + HRW*(A - B);  qq = [A+CT | A+CT]
                TT(Aj[:], Aj[:], Bj[:], sub)
                TT(Aj[:], Aj[:], HRW, mult)
                TT(Aj[:], Aj[:], Bj[:], add)
                TT(QQ[j][:, 0:128], Aj[:], CT, add)
                TT(QQ[j][:, 128:256], Aj[:], CT, add)
                # C = rel_ = REL + complex_mul(REL, q)
                # PC = [REL|RELsw]*[q|q] = [RL0q0|RL1q1 | RL1q0|RL0q1]
                PC = PB[j]
                TT(PC[:], RELRELSW, QQ[j][:], mult)
                TT(Bj[:, 0:64], PC[:, 0:64], PC[:, 64:128], add)
                TT(Bj[:, 64:128], PC[:, 192:256], PC[:, 128:192], sub)
                TT(Bj[:], Bj[:], REL, add)
                if full:
                    head_main_b(j, TT)

            def head_main_b(j, TT):
                t8 = t8v[:, j, :]
                lhs = lhsv[:, j, :]
                TM = t8[:, 256:384]
                TM0 = t8[:, 256:320]
                TM1 = t8[:, 320:384]
                TESW2 = t8[:, 384:640]
                Bj = Bt[j]
                # SS = [S|Ssw], DD = [D|D]; S = rel_+time, D = rel_-time
                TT(SS[j][:, 0:128], Bj[:], TM, add)
                TT(SS[j][:, 128:192], Bj[:, 64:128], TM1, add)
                TT(SS[j][:, 192:256], Bj[:, 0:64], TM0, add)
                TT(DD[j][:, 0:128], Bj[:], TM, sub)
                TT(DD[j][:, 128:256], Bj[:], TM, sub)
                # PL = [L|L]*[S|Ssw] = [L0S0|L1S1 | L0S1|L1S0]
                # PT = [TE|TEsw]*[D|D] = [TE0D0|TE1D1 | TE1D0|TE0D1]
                TT(PL[j][:], lhs[:, 0:256], SS[j][:], mult)
                TT(PT[j][:], TESW2, DD[j][:], mult)
                # V0 = (L0S0 - L1S1) + (TE0D0 + TE1D1)
                TT(t0[j][:], PL[j][:, 0:64], PL[j][:, 64:128], sub)
                TT(t1[j][:], PT[j][:, 0:64], PT[j][:, 64:128], add)
                TT(V[j][:, 0:64], t0[j][:], t1[j][:], add)
                # V1 = (L0S1 + L1S0) + (TE1D0 - TE0D1)
                TT(t0[j][:], PL[j][:, 128:192], PL[j][:, 192:256], add)
                TT(t1[j][:], PT[j][:, 128:192], PT[j][:, 192:256], sub)
                TT(V[j][:, 64:128], t0[j][:], t1[j][:], add)

            vts = []

            def finish_vt(j):
                vt_ps = pst.tile([P, P], dt.float16, space="PSUM", tag="vtps")
                nc.tensor.transpose(out=vt_ps[:], in_=V[j][:], identity=ident[:])
                vt = constp.tile([P, P], dt.float16, name=f"vt{j}")
                nc.scalar.copy(out=vt[:], in_=vt_ps[:])
                vts.append(vt)

            head_rule(0, GTT)
            head_main(0, VTT)
            finish_vt(0)
            # tile-1: gpsimd carries the chain through rel_; vector finishes
            # the SS/PL/PT/V assembly right after its group-0 copy
            head_rule(1, GTT)
            head_main(1, GTT)

            # ---- stream matmuls + PSUM->SBUF copies + per-1024-col OUT DMAs
            GRP = 2 * CHUNK
            osb = [constp.tile([P, NHALF], dt.float16, name=f"osb{j}")
                   for j in range(2)]
            g = 0
            for j in range(2):
                for c0 in range(0, NHALF, GRP):
                    gw = min(GRP, NHALF - c0)
                    mm = psm.tile([P, GRP], dt.float32, space="PSUM", tag="mm")
                    for lo in range(0, gw, CHUNK):
                        cw = min(CHUNK, gw - lo)
                        nc.tensor.matmul(out=mm[:, lo:lo + cw],
                                         lhsT=vts[j][:],
                                         rhs=e0t[:, c0 + lo:c0 + lo + cw],
                                         start=True, stop=True)
                    if g == 0 or (g >= 5 and g % 2 == 1):
                        nc.vector.tensor_copy(out=osb[j][:, c0:c0 + gw],
                                              in_=mm[:, :gw])
                    else:
                        nc.scalar.copy(out=osb[j][:, c0:c0 + gw],
                                       in_=mm[:, :gw])
                    if g == 16:
                        # tile-1 transpose emitted late in the stream: the
                        # tensor pipeline never parks waiting on the (slow,
                        # gpsimd-computed) tile-1 head
                        finish_vt(1)
                    g += 1
                    nc.sync.dma_start(OUT[j * P:(j + 1) * P, c0:c0 + gw],
                                      osb[j][:, c0:c0 + gw])

    nc.compile()
    return nc


def _prep_inputs(inputs):
    x = np.asarray(inputs["x"])
    E0 = np.ascontiguousarray(np.asarray(inputs["E0"], dtype=np.float32))
    E1 = np.asarray(inputs["E1"], dtype=np.float32)
    E2 = np.asarray(inputs["E2"], dtype=np.float32)
    E3 = np.asarray(inputs["E3"], dtype=np.float32)
    E4 = np.asarray(inputs["E4"], dtype=np.float32)
    E5 = np.asarray(inputs["E5"], dtype=np.float32)
    E6 = np.asarray(inputs["E6"], dtype=np.float32)
    rule_C = np.asarray(inputs["rule_C"], dtype=np.float32)
    rule_S = np.asarray(inputs["rule_S"], dtype=np.float32)
    has_rules = np.asarray(inputs["has_rules"])

    idx = np.zeros((B, 4), np.int32)
    idx[:, 0] = x[:, 1]    # r
    idx[:, 1] = x[:, 3]    # t
    idx[:, 2] = x[:, 0]    # h

    def sw(a):
        return np.concatenate([a[:, RANK // 2:], a[:, :RANK // 2]], axis=1)

    hrw = np.repeat(has_rules.astype(np.float32)[:, None], RANK, axis=1)
    rcat = np.ascontiguousarray(np.concatenate(
        [E1, sw(E1), rule_C, sw(rule_C), -rule_S[:, None] * E1,
         hrw], axis=1).astype(np.float16))
    tb = np.arange(NTIME) // CYCLE
    TM = E2 + E5[tb]
    TE = E3 + E6[tb]
    tcat = np.ascontiguousarray(np.concatenate(
        [E4, E4, TM, TE, sw(TE)], axis=1).astype(np.float16))
    e0h = E0.astype(np.float16)
    e0g = np.ascontiguousarray(np.concatenate([e0h, e0h], axis=1))
    e0t = np.ascontiguousarray(E0.T).astype(np.float16)
    e0t_halves = [np.ascontiguousarray(e0t[:, :NHALF]),
                  np.ascontiguousarray(e0t[:, NHALF:])]

    in_maps = []
    for c in range(NCORES):
        p = c // 2
        i0 = idx[2 * p * P:(2 * p + 1) * P]        # tile 0 (r,t,h,pad)
        i1 = idx[(2 * p + 1) * P:(2 * p + 2) * P]  # tile 1
        idx2 = np.empty((P, 8), np.int32)
        idx2[:, 0::2] = i0
        idx2[:, 1::2] = i1
        in_maps.append({
            "IDX": np.ascontiguousarray(idx2),
            "E0G": e0g, "RCAT": rcat, "TCAT": tcat,
            "E0T": e0t_halves[c % 2],
        })
    return in_maps


def kernel(**inputs):
    from concourse.bass_utils import run_bass_kernel_spmd

    if "nc" not in _CACHE:
        _CACHE["nc"] = _build()
    nc = _CACHE["nc"]

    in_maps = _prep_inputs(inputs)
    res = run_bass_kernel_spmd(nc, in_maps, core_ids=list(range(NCORES)),
                               trace=TRACE)
    _CACHE["last_result"] = res
    out = np.empty((B, NENT), np.float32)
    for p in range(NCORES // 2):
        lo = res.results[2 * p]["OUT"]        # [256, 0:20000]
        hi = res.results[2 * p + 1]["OUT"]    # [256, 20000:40000]
        rows = slice(2 * p * P, (2 * p + 2) * P)
        out[rows, :NHALF] = lo
        out[rows, NHALF:] = hi
    return out



# revision 10
# speedup vs baseline: 1.0058x; 1.0058x over previous
"""Trainium2 Bass kernel for nn_CTRule (temporal KG scoring model).

Computes, for each of B=1024 queries (h, r, t):
  v = f(E0[h], E1[r], time tables, rule tables)   # [B, 128] elementwise algebra
  scores = v @ E0.T                               # [B, 40000]

Distribution over the 8 NeuronCores: 2-way batch x 4-way entity grid.
Core c handles batch rows [bh*512, bh*512+512) (bh = c//4) against entity
columns [es*10000, es*10000+10000) (es = c%4).  Per-core HBM traffic is
  out 10.24 MB + E0T slice 2.56 MB + tables 1.57 MB = 14.4 MB  (~40 us at
the 358 GB/s per-core HBM limit), vs 16.2 MB and a late-starting output
stream for the previous pair-hybrid layout.

Host prep: all per-example table rows are pre-indexed on the host into one
TBL tensor ([128, 4 tiles, 1536] per core) laid out in the exact block
patterns the head algebra wants, so every complex/quaternion product is a
single wide fp16 multiply followed by a 128-wide "fold" add/sub:
  cmul(x, y)        = fold(+) of  [x0|x0|x1|x1] * [y0|y1|-y1|y0]
  complex_mul(x, y) = fold(+) of  [x0|x0|x1|-x1] * [y0|y1|y1|y0]
  mul4 tail         = fold(-/+) of Y * X1 and rev64(Y) * X1
The head is ~23 vector ops per tile group (tile 0 alone for latency, tiles
1-3 batched via [P,3,*] APs).  v is transposed on TensorE; the scores
stream through 500-col matmul chunks into [P,1000] PSUM groups, drained by
f32->fp16 casts round-robined over Scalar/GpSimd/Vector, and written out in
[128,2000] chunks on the sync ring as soon as each pair of casts lands.
No cross-core communication; the host reassembles the 8 blocks.
"""

import numpy as np

P = 128
B = 1024
RANK = 128
NENT = 40000
NTIME = 365
CYCLE = 120
NCORES = 8
ES = 4                   # entity-axis splits
BS = 2                   # batch-axis splits
NSLICE = NENT // ES      # 10000 entity columns per core
ROWS = B // BS           # 512 rows per core
NT = ROWS // P           # 4 batch tiles per core
TW = 1536                # table width per tile (see column map below)
# matmul/cast groups: [P,1024] f32 = 2 PSUM banks; chunks must be 512-col
# bank-aligned (a 500-col chunk crossing a bank boundary corrupts results).
GROUPS = [(c, 1024) for c in range(0, 9216, 1024)] + [(9216, 784)]
GRP = 1024               # first E0T chunk / group allocation width

# TBL column map (per tile):
C_RELX4 = 0      # [R0|R0|R1|-R1]           256
C_RCP = 256      # [RC0|RC1|-RC1|RC0]       256
C_CTD = 512      # [C0|C0|C1|C1]            256
C_TM = 768       # time = E2[t]+E5[tb]      128
C_TE = 896       # time_ent = E3[t]+E6[tb]  128
C_E0G = 1024     # [L0|L1|-L1|-L0]          256
C_HRW = 1280     # has_rules broadcast      128
C_HSR = 1408     # hr*rS*rel                128

TRACE = False            # set by test harness for profiling runs
_CACHE = {}


def _build():
    import concourse.bass as bass
    import concourse.mybir as mybir
    import concourse.tile as tile
    from concourse import bacc
    from concourse.masks import make_identity

    dt = mybir.dt
    mult = mybir.AluOpType.mult
    add = mybir.AluOpType.add
    sub = mybir.AluOpType.subtract

    nc = bacc.Bacc("TRN2", target_bir_lowering=False, debug=False,
                   num_devices=NCORES)

    TBL = nc.dram_tensor("TBL", [P, NT, TW], dt.float16, kind="ExternalInput").ap()
    E0T = nc.dram_tensor("E0T", [RANK, NSLICE], dt.float16, kind="ExternalInput").ap()
    OUT = nc.dram_tensor("OUT", [ROWS, NSLICE], dt.float16, kind="ExternalOutput").ap()

    def r4(ap):
        # view last dim as 4 blocks of 64
        return ap.rearrange("p t (s x) -> p t s x", s=4)

    def r2(ap):
        return ap.rearrange("p t (s x) -> p t s x", s=2)

    with tile.TileContext(nc) as tc:
        with (
            tc.tile_pool(name="const", bufs=1) as constp,
            tc.tile_pool(name="ew", bufs=1) as ew,
            tc.tile_pool(name="pst", bufs=1, space="PSUM") as pst,
            tc.tile_pool(name="psm", bufs=3, space="PSUM") as psm,
        ):
            # ---- input DMAs.  Sync ring first (earliest issue slot):
            # tile-0 tables + first E0T chunk gate the whole pipeline.
            tbl = constp.tile([P, NT, TW], dt.float16)
            e0t = constp.tile([RANK, NSLICE], dt.float16)
            nc.sync.dma_start(tbl[:, 0:1, :], TBL[:, 0:1, :])
            nc.sync.dma_start(e0t[:, 0:GRP], E0T[:, 0:GRP])
            nc.sync.dma_start(tbl[:, 1:NT, :], TBL[:, 1:NT, :])
            # bulk E0T on the Activation HWDGE ring (starts later; not critical)
            nc.scalar.dma_start(e0t[:, GRP:4000], E0T[:, GRP:4000])
            nc.scalar.dma_start(e0t[:, 4000:7000], E0T[:, 4000:7000])
            nc.scalar.dma_start(e0t[:, 7000:NSLICE], E0T[:, 7000:NSLICE])

            ident = constp.tile([P, P], dt.float16)
            make_identity(nc, ident[:])

            # ---- head: ~23 wide fp16 ops per group (VectorE or GpSimd)
            def head(tag, jj, nt, eng):
                t = tbl[:, jj, :]            # [P, nt, TW]
                T = lambda a, b: t[:, :, a:b]
                pa = ew.tile([P, nt, 256], dt.float16, name=f"pa{tag}")
                pb = ew.tile([P, nt, 256], dt.float16, name=f"pb{tag}")
                pc = ew.tile([P, nt, 256], dt.float16, name=f"pc{tag}")
                fa = ew.tile([P, nt, 128], dt.float16, name=f"fa{tag}")
                bt = ew.tile([P, nt, 128], dt.float16, name=f"bt{tag}")
                bc = ew.tile([P, nt, 128], dt.float16, name=f"bc{tag}")
                g = ew.tile([P, nt, 128], dt.float16, name=f"g{tag}")
                w2 = ew.tile([P, nt, 256], dt.float16, name=f"w2{tag}")
                fc = ew.tile([P, nt, 128], dt.float16, name=f"fc{tag}")
                yy = ew.tile([P, nt, 256], dt.float16, name=f"yy{tag}")
                x1 = ew.tile([P, nt, 256], dt.float16, name=f"x1{tag}")
                ma = ew.tile([P, nt, 256], dt.float16, name=f"ma{tag}")
                mb = ew.tile([P, nt, 256], dt.float16, name=f"mb{tag}")
                fm = ew.tile([P, nt, 128], dt.float16, name=f"fm{tag}")
                fn = ew.tile([P, nt, 128], dt.float16, name=f"fn{tag}")
                vv = ew.tile([P, nt, 128], dt.float16, name=f"vv{tag}")

                def TT(out, a, b, op):
                    eng.tensor_tensor(out=out, in0=a, in1=b, op=op)

                # rule branch: fa = cmul(CT, RC)
                TT(pa[:], T(C_CTD, C_CTD + 256), T(C_RCP, C_RCP + 256), mult)
                TT(fa[:], pa[:, :, 0:128], pa[:, :, 128:256], add)
                # no-rule branch: bt = lhs + cmul(rel, lhs)
                TT(pb[:], T(C_RELX4, C_RELX4 + 256), T(C_E0G, C_E0G + 256), mult)
                TT(bt[:], pb[:, :, 0:128], pb[:, :, 128:256], add)
                TT(bt[:], bt[:], T(C_E0G, C_E0G + 128), add)
                # bc = bt + CT  (CT = blocks {0,2} of CTdup)
                TT(r2(bc[:]), r2(bt[:]), r4(T(C_CTD, C_CTD + 256))[:, :, 0::2, :], add)
                # w = hr*(fa - rS*rel - bt) + bt + CT
                TT(g[:], fa[:], bt[:], sub)
                TT(g[:], g[:], T(C_HRW, C_HRW + 128), mult)
                TT(g[:], g[:], T(C_HSR, C_HSR + 128), sub)
                TT(w2[:, :, 0:128], g[:], bc[:], add)
                eng.tensor_copy(out=r2(w2[:, :, 128:256]),
                                in_=r2(w2[:, :, 0:128])[:, :, ::-1, :])
                # rel_ = rel + complex_mul(rel, w) -> Y blocks {0,2}
                TT(pc[:], T(C_RELX4, C_RELX4 + 256), w2[:], mult)
                TT(fc[:], pc[:, :, 0:128], pc[:, :, 128:256], add)
                TT(r4(yy[:])[:, :, 0::2, :], r2(fc[:]),
                   r4(T(C_RELX4, C_RELX4 + 256))[:, :, 0::2, :], add)
                # Y blocks {1,3} = TM halves
                eng.tensor_copy(out=r4(yy[:])[:, :, 1::2, :],
                                in_=r2(T(C_TM, C_TM + 128)))
                # X1 = [L0+T0 | L0-T0 | L1-T1 | L1+T1]
                TT(r4(x1[:])[:, :, 0::3, :], r2(T(C_E0G, C_E0G + 128)),
                   r2(T(C_TE, C_TE + 128)), add)
                TT(r4(x1[:])[:, :, 1:3, :], r2(T(C_E0G, C_E0G + 128)),
                   r2(T(C_TE, C_TE + 128)), sub)
                # v
                TT(ma[:], yy[:], x1[:], mult)
                TT(mb[:], r4(yy[:])[:, :, ::-1, :], x1[:], mult)
                TT(fm[:], ma[:, :, 0:128], ma[:, :, 128:256], sub)
                TT(vv[:, :, 0:64], fm[:, :, 0:64], fm[:, :, 64:128], add)
                TT(fn[:], mb[:, :, 0:128], mb[:, :, 128:256], add)
                TT(vv[:, :, 64:128], fn[:, :, 0:64], fn[:, :, 64:128], add)
                return vv

            v0 = head(0, slice(0, 1), 1, nc.vector)
            # tile 3's head runs concurrently on GpSimd (slow engine, but
            # vt3 isn't needed until its matmuls ~t+25us)
            v3 = head(3, slice(3, 4), 1, nc.gpsimd)
            vts = []

            def finish_vt(vsrc, k):
                vt_ps = pst.tile([P, P], dt.float16, space="PSUM", tag="vtps")
                nc.tensor.transpose(out=vt_ps[:], in_=vsrc[:, k, :],
                                    identity=ident[:])
                vt = constp.tile([P, P], dt.float16, name=f"vt{len(vts)}")
                nc.scalar.copy(out=vt[:], in_=vt_ps[:])
                vts.append(vt)

            finish_vt(v0, 0)
            v12 = head(1, slice(1, 3), 2, nc.vector)

            osb = [constp.tile([P, NSLICE], dt.float16, name=f"osb{i}")
                   for i in range(NT)]
            cast_cnt = [0]

            def cast(dst, src, engine):
                # GPSIMD cannot read PSUM; only Scalar(ACT)/Vector can drain
                if engine == 0:
                    nc.scalar.copy(out=dst, in_=src)
                else:
                    nc.vector.tensor_copy(out=dst, in_=src)

            for j in range(NT):
                if j == 1:
                    finish_vt(v12, 0)
                    finish_vt(v12, 1)
                    finish_vt(v3, 0)
                ob = osb[j]
                for gi, (c0, gw) in enumerate(GROUPS):
                    mm = psm.tile([P, 1024], dt.float32, space="PSUM", tag="mm")
                    for lo in range(0, gw, 512):
                        cw = min(512, gw - lo)
                        nc.tensor.matmul(out=mm[:, lo:lo + cw],
                                         lhsT=vts[j][:],
                                         rhs=e0t[:, c0 + lo:c0 + lo + cw],
                                         start=True, stop=True)
                    eng = cast_cnt[0] % 2   # alternate scalar/vector
                    cast_cnt[0] += 1
                    cast(ob[:, c0:c0 + gw], mm[:, 0:gw], eng)
                    if gi % 2 == 1:
                        oc, ow = GROUPS[gi - 1][0], GROUPS[gi - 1][1] + gw
                        nc.sync.dma_start(
                            OUT[j * P:(j + 1) * P, oc:oc + ow],
                            ob[:, oc:oc + ow])

    nc.compile()
    return nc


def _prep_inputs(inputs):
    x = np.asarray(inputs["x"])
    E0 = np.asarray(inputs["E0"], dtype=np.float32)
    E1 = np.asarray(inputs["E1"], dtype=np.float32)
    E2 = np.asarray(inputs["E2"], dtype=np.float32)
    E3 = np.asarray(inputs["E3"], dtype=np.float32)
    E4 = np.asarray(inputs["E4"], dtype=np.float32)
    E5 = np.asarray(inputs["E5"], dtype=np.float32)
    E6 = np.asarray(inputs["E6"], dtype=np.float32)
    rule_C = np.asarray(inputs["rule_C"], dtype=np.float32)
    rule_S = np.asarray(inputs["rule_S"], dtype=np.float32)
    has_rules = np.asarray(inputs["has_rules"])

    h, r, t = x[:, 0].astype(np.int64), x[:, 1].astype(np.int64), x[:, 3].astype(np.int64)
    tb = t // CYCLE
    H = RANK // 2

    L = E0[h]
    R = E1[r]
    RC = rule_C[r]
    CT = E4[t]
    TM = E2[t] + E5[tb]
    TE = E3[t] + E6[tb]
    hr = has_rules[r].astype(np.float32)[:, None]
    hsr = hr * rule_S[r][:, None]

    def hs(a):
        return a[:, :H], a[:, H:]

    L0, L1 = hs(L)
    R0, R1 = hs(R)
    RC0, RC1 = hs(RC)
    C0, C1 = hs(CT)

    tblex = np.concatenate([
        R0, R0, R1, -R1,          # RELX4
        RC0, RC1, -RC1, RC0,      # RCP
        C0, C0, C1, C1,           # CTdup
        TM, TE,
        L0, L1, -L1, -L0,         # E0GX
        np.repeat(hr, RANK, axis=1),
        hsr * R,
    ], axis=1).astype(np.float16)   # [B, TW]
    assert tblex.shape[1] == TW

    e0t = np.ascontiguousarray(E0.T.astype(np.float16))   # [128, 40000]

    tbl_by_bh = []
    for bh in range(BS):
        rows = tblex[bh * ROWS:(bh + 1) * ROWS]
        tbl_by_bh.append(np.ascontiguousarray(
            rows.reshape(NT, P, TW).transpose(1, 0, 2)))
    e0t_by_es = [np.ascontiguousarray(e0t[:, es * NSLICE:(es + 1) * NSLICE])
                 for es in range(ES)]

    in_maps = []
    for c in range(NCORES):
        in_maps.append({
            "TBL": tbl_by_bh[c // ES],
            "E0T": e0t_by_es[c % ES],
        })
    return in_maps


def kernel(**inputs):
    from concourse.bass_utils import run_bass_kernel_spmd

    if "nc" not in _CACHE:
        _CACHE["nc"] = _build()
    nc = _CACHE["nc"]

    in_maps = _prep_inputs(inputs)
    res = run_bass_kernel_spmd(nc, in_maps, core_ids=list(range(NCORES)),
                               trace=TRACE)
    _CACHE["last_result"] = res
    out = np.empty((B, NENT), np.float32)
    for c in range(NCORES):
        bh, es = c // ES, c % ES
        out[bh * ROWS:(bh + 1) * ROWS,
            es * NSLICE:(es + 1) * NSLICE] = res.results[c]["OUT"]
    return out


# revision 11
# speedup vs baseline: 1.1077x; 1.1013x over previous
"""Trainium2 Bass kernel for nn_CTRule (temporal KG scoring model).

Computes, for each of B=1024 queries (h, r, t):
  v = f(E0[h], E1[r], time tables, rule tables)   # [B, 128] elementwise algebra
  scores = v @ E0.T                               # [B, 40000]

Distribution over the 8 NeuronCores: 2-way batch x 4-way entity grid.
Core c handles batch rows [bh*512, bh*512+512) (bh = c//4) against entity
columns [es*10000, es*10000+10000) (es = c%4).  Per-core HBM traffic is
  out 10.24 MB + E0T slice 2.56 MB + tables 1.57 MB = 14.4 MB  (~40 us at
the 358 GB/s per-core HBM limit), vs 16.2 MB and a late-starting output
stream for the previous pair-hybrid layout.

Host prep: all per-example table rows are pre-indexed on the host into one
TBL tensor ([128, 4 tiles, 1536] per core) laid out in the exact block
patterns the head algebra wants, so every complex/quaternion product is a
single wide fp16 multiply followed by a 128-wide "fold" add/sub:
  cmul(x, y)        = fold(+) of  [x0|x0|x1|x1] * [y0|y1|-y1|y0]
  complex_mul(x, y) = fold(+) of  [x0|x0|x1|-x1] * [y0|y1|y1|y0]
  mul4 tail         = fold(-/+) of Y * X1 and rev64(Y) * X1
The head is ~23 vector ops per tile group (tile 0 alone for latency, tiles
1-3 batched via [P,3,*] APs).  v is transposed on TensorE; the scores
stream through 500-col matmul chunks into [P,1000] PSUM groups, drained by
f32->fp16 casts round-robined over Scalar/GpSimd/Vector, and written out in
[128,2000] chunks on the sync ring as soon as each pair of casts lands.
No cross-core communication; the host reassembles the 8 blocks.
"""

import numpy as np

P = 128
B = 1024
RANK = 128
NENT = 40000
NTIME = 365
CYCLE = 120
NCORES = 8
ES = 4                   # entity-axis splits
BS = 2                   # batch-axis splits
NSLICE = NENT // ES      # 10000 entity columns per core
ROWS = B // BS           # 512 rows per core
NT = ROWS // P           # 4 batch tiles per core
TW = 1536                # table width per tile (see column map below)
# matmul/cast groups: [P,1024] f32 = 2 PSUM banks; chunks must be 512-col
# bank-aligned (a 500-col chunk crossing a bank boundary corrupts results).
GROUPS = [(c, 1024) for c in range(0, 9216, 1024)] + [(9216, 784)]
GRP = 1024               # first E0T chunk / group allocation width

# TBL column map (per tile):
C_RELX4 = 0      # [R0|R0|R1|-R1]           256
C_RCP = 256      # [RC0|RC1|-RC1|RC0]       256
C_CTD = 512      # [C0|C0|C1|C1]            256
C_TM = 768       # time = E2[t]+E5[tb]      128
C_TE = 896       # time_ent = E3[t]+E6[tb]  128
C_E0G = 1024     # [L0|L1|-L1|-L0]          256
C_HRW = 1280     # has_rules broadcast      128
C_HSR = 1408     # hr*rS*rel                128

TRACE = False            # set by test harness for profiling runs
_CACHE = {}


def _build():
    import concourse.bass as bass
    import concourse.mybir as mybir
    import concourse.tile as tile
    from concourse import bacc

    dt = mybir.dt
    mult = mybir.AluOpType.mult
    add = mybir.AluOpType.add
    sub = mybir.AluOpType.subtract

    nc = bacc.Bacc("TRN2", target_bir_lowering=False, debug=False,
                   num_devices=NCORES)

    TBL = nc.dram_tensor("TBL", [P, NT, TW], dt.float16, kind="ExternalInput").ap()
    E0T = nc.dram_tensor("E0T", [RANK, NSLICE], dt.float16, kind="ExternalInput").ap()
    IDN = nc.dram_tensor("IDN", [P, P], dt.float16, kind="ExternalInput").ap()
    OUT = nc.dram_tensor("OUT", [ROWS, NSLICE], dt.float16, kind="ExternalOutput").ap()

    def r4(ap):
        # view last dim as 4 blocks of 64
        return ap.rearrange("p t (s x) -> p t s x", s=4)

    def r2(ap):
        return ap.rearrange("p t (s x) -> p t s x", s=2)

    with tile.TileContext(nc) as tc:
        with (
            tc.tile_pool(name="const", bufs=1) as constp,
            tc.tile_pool(name="ew", bufs=1) as ew,
            tc.tile_pool(name="pst", bufs=1, space="PSUM") as pst,
            tc.tile_pool(name="psm", bufs=3, space="PSUM") as psm,
        ):
            # ---- input DMAs.  Sync ring first (earliest issue slot), in
            # dependency-latency order.  Separate tbl tiles per head group so
            # each group's first op waits only on its own DMA.
            tbl0 = constp.tile([P, 1, TW], dt.float16, name="tbl0")
            tbl12 = constp.tile([P, 2, TW], dt.float16, name="tbl12")
            tbl3 = constp.tile([P, 1, TW], dt.float16, name="tbl3")
            e0t = constp.tile([RANK, NSLICE], dt.float16)
            ident = constp.tile([P, P], dt.float16)
            nc.sync.dma_start(tbl0[:], TBL[:, 0:1, :])
            nc.sync.dma_start(ident[:], IDN[:])
            nc.sync.dma_start(e0t[:, 0:GRP], E0T[:, 0:GRP])
            nc.sync.dma_start(tbl12[:], TBL[:, 1:3, :])
            nc.sync.dma_start(tbl3[:], TBL[:, 3:4, :])
            # bulk E0T on the Activation HWDGE ring (starts later; not critical)
            nc.scalar.dma_start(e0t[:, GRP:4000], E0T[:, GRP:4000])
            nc.scalar.dma_start(e0t[:, 4000:7000], E0T[:, 4000:7000])
            nc.scalar.dma_start(e0t[:, 7000:NSLICE], E0T[:, 7000:NSLICE])

            # ---- head: ~23 wide fp16 ops per group (VectorE or GpSimd)
            def head(tag, tsrc, nt, eng):
                t = tsrc[:]                  # [P, nt, TW]
                T = lambda a, b: t[:, :, a:b]
                pa = ew.tile([P, nt, 256], dt.float16, name=f"pa{tag}")
                pb = ew.tile([P, nt, 256], dt.float16, name=f"pb{tag}")
                pc = ew.tile([P, nt, 256], dt.float16, name=f"pc{tag}")
                fa = ew.tile([P, nt, 128], dt.float16, name=f"fa{tag}")
                bt = ew.tile([P, nt, 128], dt.float16, name=f"bt{tag}")
                bc = ew.tile([P, nt, 128], dt.float16, name=f"bc{tag}")
                g = ew.tile([P, nt, 128], dt.float16, name=f"g{tag}")
                w2 = ew.tile([P, nt, 256], dt.float16, name=f"w2{tag}")
                fc = ew.tile([P, nt, 128], dt.float16, name=f"fc{tag}")
                yy = ew.tile([P, nt, 256], dt.float16, name=f"yy{tag}")
                x1 = ew.tile([P, nt, 256], dt.float16, name=f"x1{tag}")
                ma = ew.tile([P, nt, 256], dt.float16, name=f"ma{tag}")
                mb = ew.tile([P, nt, 256], dt.float16, name=f"mb{tag}")
                fm = ew.tile([P, nt, 128], dt.float16, name=f"fm{tag}")
                fn = ew.tile([P, nt, 128], dt.float16, name=f"fn{tag}")
                vv = ew.tile([P, nt, 128], dt.float16, name=f"vv{tag}")

                def TT(out, a, b, op):
                    eng.tensor_tensor(out=out, in0=a, in1=b, op=op)

                # rule branch: fa = cmul(CT, RC)
                TT(pa[:], T(C_CTD, C_CTD + 256), T(C_RCP, C_RCP + 256), mult)
                TT(fa[:], pa[:, :, 0:128], pa[:, :, 128:256], add)
                # no-rule branch: bt = lhs + cmul(rel, lhs)
                TT(pb[:], T(C_RELX4, C_RELX4 + 256), T(C_E0G, C_E0G + 256), mult)
                TT(bt[:], pb[:, :, 0:128], pb[:, :, 128:256], add)
                TT(bt[:], bt[:], T(C_E0G, C_E0G + 128), add)
                # bc = bt + CT  (CT = blocks {0,2} of CTdup)
                TT(r2(bc[:]), r2(bt[:]), r4(T(C_CTD, C_CTD + 256))[:, :, 0::2, :], add)
                # w = hr*(fa - rS*rel - bt) + bt + CT
                TT(g[:], fa[:], bt[:], sub)
                TT(g[:], g[:], T(C_HRW, C_HRW + 128), mult)
                TT(g[:], g[:], T(C_HSR, C_HSR + 128), sub)
                TT(w2[:, :, 0:128], g[:], bc[:], add)
                eng.tensor_copy(out=r2(w2[:, :, 128:256]),
                                in_=r2(w2[:, :, 0:128])[:, :, ::-1, :])
                # rel_ = rel + complex_mul(rel, w) -> Y blocks {0,2}
                TT(pc[:], T(C_RELX4, C_RELX4 + 256), w2[:], mult)
                TT(fc[:], pc[:, :, 0:128], pc[:, :, 128:256], add)
                TT(r4(yy[:])[:, :, 0::2, :], r2(fc[:]),
                   r4(T(C_RELX4, C_RELX4 + 256))[:, :, 0::2, :], add)
                # Y blocks {1,3} = TM halves
                eng.tensor_copy(out=r4(yy[:])[:, :, 1::2, :],
                                in_=r2(T(C_TM, C_TM + 128)))
                # X1 = [L0+T0 | L0-T0 | L1-T1 | L1+T1]
                TT(r4(x1[:])[:, :, 0::3, :], r2(T(C_E0G, C_E0G + 128)),
                   r2(T(C_TE, C_TE + 128)), add)
                TT(r4(x1[:])[:, :, 1:3, :], r2(T(C_E0G, C_E0G + 128)),
                   r2(T(C_TE, C_TE + 128)), sub)
                # v
                TT(ma[:], yy[:], x1[:], mult)
                TT(mb[:], r4(yy[:])[:, :, ::-1, :], x1[:], mult)
                TT(fm[:], ma[:, :, 0:128], ma[:, :, 128:256], sub)
                TT(vv[:, :, 0:64], fm[:, :, 0:64], fm[:, :, 64:128], add)
                TT(fn[:], mb[:, :, 0:128], mb[:, :, 128:256], add)
                TT(vv[:, :, 64:128], fn[:, :, 0:64], fn[:, :, 64:128], add)
                return vv

            v0 = head(0, tbl0, 1, nc.vector)
            # tile 3's head runs concurrently on GpSimd (slow engine, but
            # vt3 isn't needed until its matmuls ~t+25us)
            v3 = head(3, tbl3, 1, nc.gpsimd)
            vts = []

            def finish_vt(vsrc, k):
                vt_ps = pst.tile([P, P], dt.float16, space="PSUM", tag="vtps")
                nc.tensor.transpose(out=vt_ps[:], in_=vsrc[:, k, :],
                                    identity=ident[:])
                vt = constp.tile([P, P], dt.float16, name=f"vt{len(vts)}")
                nc.scalar.copy(out=vt[:], in_=vt_ps[:])
                vts.append(vt)

            finish_vt(v0, 0)
            v12 = head(1, tbl12, 2, nc.vector)

            osb = [constp.tile([P, NSLICE], dt.float16, name=f"osb{i}")
                   for i in range(NT)]
            cast_cnt = [0]

            def cast(dst, src, engine):
                # GPSIMD cannot read PSUM; only Scalar(ACT)/Vector can drain
                if engine == 0:
                    nc.scalar.copy(out=dst, in_=src)
                else:
                    nc.vector.tensor_copy(out=dst, in_=src)

            for j in range(NT):
                if j == 1:
                    finish_vt(v12, 0)
                    finish_vt(v12, 1)
                    finish_vt(v3, 0)
                ob = osb[j]
                for gi, (c0, gw) in enumerate(GROUPS):
                    mm = psm.tile([P, 1024], dt.float32, space="PSUM", tag="mm")
                    for lo in range(0, gw, 512):
                        cw = min(512, gw - lo)
                        nc.tensor.matmul(out=mm[:, lo:lo + cw],
                                         lhsT=vts[j][:],
                                         rhs=e0t[:, c0 + lo:c0 + lo + cw],
                                         start=True, stop=True)
                    if j == 0:
                        eng = 0   # tile 0: scalar only (vector still on heads)
                    else:
                        eng = 1 - (cast_cnt[0] % 2)   # alternate vector/scalar
                        cast_cnt[0] += 1
                    cast(ob[:, c0:c0 + gw], mm[:, 0:gw], eng)
                    if gi % 2 == 1:
                        oc, ow = GROUPS[gi - 1][0], GROUPS[gi - 1][1] + gw
                        nc.sync.dma_start(
                            OUT[j * P:(j + 1) * P, oc:oc + ow],
                            ob[:, oc:oc + ow])

    nc.compile()
    return nc


def _prep_inputs(inputs):
    x = np.asarray(inputs["x"])
    E0 = np.asarray(inputs["E0"], dtype=np.float32)
    E1 = np.asarray(inputs["E1"], dtype=np.float32)
    E2 = np.asarray(inputs["E2"], dtype=np.float32)
    E3 = np.asarray(inputs["E3"], dtype=np.float32)
    E4 = np.asarray(inputs["E4"], dtype=np.float32)
    E5 = np.asarray(inputs["E5"], dtype=np.float32)
    E6 = np.asarray(inputs["E6"], dtype=np.float32)
    rule_C = np.asarray(inputs["rule_C"], dtype=np.float32)
    rule_S = np.asarray(inputs["rule_S"], dtype=np.float32)
    has_rules = np.asarray(inputs["has_rules"])

    h, r, t = x[:, 0].astype(np.int64), x[:, 1].astype(np.int64), x[:, 3].astype(np.int64)
    tb = t // CYCLE
    H = RANK // 2

    L = E0[h]
    R = E1[r]
    RC = rule_C[r]
    CT = E4[t]
    TM = E2[t] + E5[tb]
    TE = E3[t] + E6[tb]
    hr = has_rules[r].astype(np.float32)[:, None]
    hsr = hr * rule_S[r][:, None]

    def hs(a):
        return a[:, :H], a[:, H:]

    L0, L1 = hs(L)
    R0, R1 = hs(R)
    RC0, RC1 = hs(RC)
    C0, C1 = hs(CT)

    tblex = np.concatenate([
        R0, R0, R1, -R1,          # RELX4
        RC0, RC1, -RC1, RC0,      # RCP
        C0, C0, C1, C1,           # CTdup
        TM, TE,
        L0, L1, -L1, -L0,         # E0GX
        np.repeat(hr, RANK, axis=1),
        hsr * R,
    ], axis=1).astype(np.float16)   # [B, TW]
    assert tblex.shape[1] == TW

    e0t = np.ascontiguousarray(E0.T.astype(np.float16))   # [128, 40000]

    tbl_by_bh = []
    for bh in range(BS):
        rows = tblex[bh * ROWS:(bh + 1) * ROWS]
        tbl_by_bh.append(np.ascontiguousarray(
            rows.reshape(NT, P, TW).transpose(1, 0, 2)))
    e0t_by_es = [np.ascontiguousarray(e0t[:, es * NSLICE:(es + 1) * NSLICE])
                 for es in range(ES)]

    ident = np.eye(P, dtype=np.float16)
    in_maps = []
    for c in range(NCORES):
        in_maps.append({
            "TBL": tbl_by_bh[c // ES],
            "E0T": e0t_by_es[c % ES],
            "IDN": ident,
        })
    return in_maps


def kernel(**inputs):
    from concourse.bass_utils import run_bass_kernel_spmd

    if "nc" not in _CACHE:
        _CACHE["nc"] = _build()
    nc = _CACHE["nc"]

    in_maps = _prep_inputs(inputs)
    res = run_bass_kernel_spmd(nc, in_maps, core_ids=list(range(NCORES)),
                               trace=TRACE)
    _CACHE["last_result"] = res
    out = np.empty((B, NENT), np.float32)
    for c in range(NCORES):
        bh, es = c // ES, c % ES
        out[bh * ROWS:(bh + 1) * ROWS,
            es * NSLICE:(es + 1) * NSLICE] = res.results[c]["OUT"]
    return out


# revision 12
# speedup vs baseline: 1.2122x; 1.0943x over previous
"""Trainium2 Bass kernel for nn_CTRule (temporal KG scoring model).

Computes, for each of B=1024 queries (h, r, t):
  v = f(E0[h], E1[r], time tables, rule tables)   # [B, 128] elementwise algebra
  scores = v @ E0.T                               # [B, 40000]

Distribution over the 8 NeuronCores: 2-way batch x 4-way entity grid.
Core c handles batch rows [bh*512, bh*512+512) (bh = c//4) against entity
columns [es*10000, es*10000+10000) (es = c%4).  Per-core HBM traffic is
  out 10.24 MB + E0T slice 2.56 MB + tables 1.57 MB = 14.4 MB  (~40 us at
the 358 GB/s per-core HBM limit), vs 16.2 MB and a late-starting output
stream for the previous pair-hybrid layout.

Host prep: all per-example table rows are pre-indexed on the host into one
TBL tensor ([128, 4 tiles, 1536] per core) laid out in the exact block
patterns the head algebra wants, so every complex/quaternion product is a
single wide fp16 multiply followed by a 128-wide "fold" add/sub:
  cmul(x, y)        = fold(+) of  [x0|x0|x1|x1] * [y0|y1|-y1|y0]
  complex_mul(x, y) = fold(+) of  [x0|x0|x1|-x1] * [y0|y1|y1|y0]
  mul4 tail         = fold(-/+) of Y * X1 and rev64(Y) * X1
The head is ~23 vector ops per tile group (tile 0 alone for latency, tiles
1-3 batched via [P,3,*] APs).  v is transposed on TensorE; the scores
stream through 500-col matmul chunks into [P,1000] PSUM groups, drained by
f32->fp16 casts round-robined over Scalar/GpSimd/Vector, and written out in
[128,2000] chunks on the sync ring as soon as each pair of casts lands.
No cross-core communication; the host reassembles the 8 blocks.
"""

import numpy as np

P = 128
B = 1024
RANK = 128
NENT = 40000
NTIME = 365
CYCLE = 120
NCORES = 8
ES = 4                   # entity-axis splits
BS = 2                   # batch-axis splits
NSLICE = NENT // ES      # 10000 entity columns per core
ROWS = B // BS           # 512 rows per core
NT = ROWS // P           # 4 batch tiles per core
TW = 1536                # table width per tile (see column map below)
# matmul/cast groups: [P,1024] f32 = 2 PSUM banks; chunks must be 512-col
# bank-aligned (a 500-col chunk crossing a bank boundary corrupts results).
GROUPS = [(c, 1024) for c in range(0, 9216, 1024)] + [(9216, 784)]
GRP = 1024               # first E0T chunk / group allocation width

# TBL column map (per tile):
C_RELX4 = 0      # [R0|R0|R1|-R1]           256
C_RCP = 256      # [RC0|RC1|-RC1|RC0]       256
C_CTD = 512      # [C0|C0|C1|C1]            256
C_TM = 768       # time = E2[t]+E5[tb]      128
C_TE = 896       # time_ent = E3[t]+E6[tb]  128
C_E0G = 1024     # [L0|L1|-L1|-L0]          256
C_HRW = 1280     # has_rules broadcast      128
C_HSR = 1408     # hr*rS*rel                128

TRACE = False            # set by test harness for profiling runs
_CACHE = {}


def _build():
    import concourse.bass as bass
    import concourse.mybir as mybir
    import concourse.tile as tile
    from concourse import bacc

    dt = mybir.dt
    mult = mybir.AluOpType.mult
    add = mybir.AluOpType.add
    sub = mybir.AluOpType.subtract

    nc = bacc.Bacc("TRN2", target_bir_lowering=False, debug=False,
                   num_devices=NCORES)

    TBL = nc.dram_tensor("TBL", [P, NT, TW], dt.float16, kind="ExternalInput").ap()
    E0T = nc.dram_tensor("E0T", [RANK, NSLICE], dt.float16, kind="ExternalInput").ap()
    IDN = nc.dram_tensor("IDN", [P, P], dt.float16, kind="ExternalInput").ap()
    OUT = nc.dram_tensor("OUT", [ROWS, NSLICE], dt.float16, kind="ExternalOutput").ap()

    def r4(ap):
        # view last dim as 4 blocks of 64
        return ap.rearrange("p t (s x) -> p t s x", s=4)

    def r2(ap):
        return ap.rearrange("p t (s x) -> p t s x", s=2)

    with tile.TileContext(nc) as tc:
        with (
            tc.tile_pool(name="const", bufs=1) as constp,
            tc.tile_pool(name="ew", bufs=1) as ew,
            tc.tile_pool(name="pst", bufs=1, space="PSUM") as pst,
            tc.tile_pool(name="psm", bufs=3, space="PSUM") as psm,
        ):
            # ---- input DMAs.  Sync ring first (earliest issue slot), in
            # dependency-latency order.  Separate tbl tiles per head group so
            # each group's first op waits only on its own DMA.
            tbl0 = constp.tile([P, 1, TW], dt.float16, name="tbl0")
            tbl12 = constp.tile([P, 2, TW], dt.float16, name="tbl12")
            tbl3 = constp.tile([P, 1, TW], dt.float16, name="tbl3")
            e0t = constp.tile([RANK, NSLICE], dt.float16)
            ident = constp.tile([P, P], dt.float16)
            # Single sync-ring FIFO for ALL input DMAs: both HWDGE rings
            # share the 16 SDMA engines round-robin, so splitting inputs
            # across rings delays the latency-critical table loads.  FIFO
            # order = dependency-latency order; OUT chunks follow behind.
            nc.sync.dma_start(tbl0[:], TBL[:, 0:1, :])
            nc.sync.dma_start(ident[:], IDN[:])
            nc.sync.dma_start(e0t[:, 0:GRP], E0T[:, 0:GRP])
            nc.sync.dma_start(tbl12[:], TBL[:, 1:3, :])
            nc.sync.dma_start(tbl3[:], TBL[:, 3:4, :])
            nc.sync.dma_start(e0t[:, GRP:4096], E0T[:, GRP:4096])
            nc.sync.dma_start(e0t[:, 4096:7168], E0T[:, 4096:7168])
            nc.sync.dma_start(e0t[:, 7168:NSLICE], E0T[:, 7168:NSLICE])

            # ---- head: ~23 wide fp16 ops per group (VectorE or GpSimd)
            def head(tag, tsrc, nt, eng):
                t = tsrc[:]                  # [P, nt, TW]
                T = lambda a, b: t[:, :, a:b]
                pa = ew.tile([P, nt, 256], dt.float16, name=f"pa{tag}")
                pb = ew.tile([P, nt, 256], dt.float16, name=f"pb{tag}")
                pc = ew.tile([P, nt, 256], dt.float16, name=f"pc{tag}")
                fa = ew.tile([P, nt, 128], dt.float16, name=f"fa{tag}")
                bt = ew.tile([P, nt, 128], dt.float16, name=f"bt{tag}")
                bc = ew.tile([P, nt, 128], dt.float16, name=f"bc{tag}")
                g = ew.tile([P, nt, 128], dt.float16, name=f"g{tag}")
                w2 = ew.tile([P, nt, 256], dt.float16, name=f"w2{tag}")
                fc = ew.tile([P, nt, 128], dt.float16, name=f"fc{tag}")
                yy = ew.tile([P, nt, 256], dt.float16, name=f"yy{tag}")
                x1 = ew.tile([P, nt, 256], dt.float16, name=f"x1{tag}")
                ma = ew.tile([P, nt, 256], dt.float16, name=f"ma{tag}")
                mb = ew.tile([P, nt, 256], dt.float16, name=f"mb{tag}")
                fm = ew.tile([P, nt, 128], dt.float16, name=f"fm{tag}")
                fn = ew.tile([P, nt, 128], dt.float16, name=f"fn{tag}")
                vv = ew.tile([P, nt, 128], dt.float16, name=f"vv{tag}")

                def TT(out, a, b, op):
                    eng.tensor_tensor(out=out, in0=a, in1=b, op=op)

                # rule branch: fa = cmul(CT, RC)
                TT(pa[:], T(C_CTD, C_CTD + 256), T(C_RCP, C_RCP + 256), mult)
                TT(fa[:], pa[:, :, 0:128], pa[:, :, 128:256], add)
                # no-rule branch: bt = lhs + cmul(rel, lhs)
                TT(pb[:], T(C_RELX4, C_RELX4 + 256), T(C_E0G, C_E0G + 256), mult)
                TT(bt[:], pb[:, :, 0:128], pb[:, :, 128:256], add)
                TT(bt[:], bt[:], T(C_E0G, C_E0G + 128), add)
                # bc = bt + CT  (CT = blocks {0,2} of CTdup)
                TT(r2(bc[:]), r2(bt[:]), r4(T(C_CTD, C_CTD + 256))[:, :, 0::2, :], add)
                # w = hr*(fa - rS*rel - bt) + bt + CT
                TT(g[:], fa[:], bt[:], sub)
                TT(g[:], g[:], T(C_HRW, C_HRW + 128), mult)
                TT(g[:], g[:], T(C_HSR, C_HSR + 128), sub)
                TT(w2[:, :, 0:128], g[:], bc[:], add)
                eng.tensor_copy(out=r2(w2[:, :, 128:256]),
                                in_=r2(w2[:, :, 0:128])[:, :, ::-1, :])
                # rel_ = rel + complex_mul(rel, w) -> Y blocks {0,2}
                TT(pc[:], T(C_RELX4, C_RELX4 + 256), w2[:], mult)
                TT(fc[:], pc[:, :, 0:128], pc[:, :, 128:256], add)
                TT(r4(yy[:])[:, :, 0::2, :], r2(fc[:]),
                   r4(T(C_RELX4, C_RELX4 + 256))[:, :, 0::2, :], add)
                # Y blocks {1,3} = TM halves
                eng.tensor_copy(out=r4(yy[:])[:, :, 1::2, :],
                                in_=r2(T(C_TM, C_TM + 128)))
                # X1 = [L0+T0 | L0-T0 | L1-T1 | L1+T1]
                TT(r4(x1[:])[:, :, 0::3, :], r2(T(C_E0G, C_E0G + 128)),
                   r2(T(C_TE, C_TE + 128)), add)
                TT(r4(x1[:])[:, :, 1:3, :], r2(T(C_E0G, C_E0G + 128)),
                   r2(T(C_TE, C_TE + 128)), sub)
                # v
                TT(ma[:], yy[:], x1[:], mult)
                TT(mb[:], r4(yy[:])[:, :, ::-1, :], x1[:], mult)
                TT(fm[:], ma[:, :, 0:128], ma[:, :, 128:256], sub)
                TT(vv[:, :, 0:64], fm[:, :, 0:64], fm[:, :, 64:128], add)
                TT(fn[:], mb[:, :, 0:128], mb[:, :, 128:256], add)
                TT(vv[:, :, 64:128], fn[:, :, 0:64], fn[:, :, 64:128], add)
                return vv

            v0 = head(0, tbl0, 1, nc.vector)
            # tile 3's head runs concurrently on GpSimd (slow engine, but
            # vt3 isn't needed until its matmuls ~t+25us)
            v3 = head(3, tbl3, 1, nc.gpsimd)
            vts = []

            def finish_vt(vsrc, k):
                vt_ps = pst.tile([P, P], dt.float16, space="PSUM", tag="vtps")
                nc.tensor.transpose(out=vt_ps[:], in_=vsrc[:, k, :],
                                    identity=ident[:])
                vt = constp.tile([P, P], dt.float16, name=f"vt{len(vts)}")
                nc.scalar.copy(out=vt[:], in_=vt_ps[:])
                vts.append(vt)

            finish_vt(v0, 0)
            v12 = head(1, tbl12, 2, nc.vector)

            osb = [constp.tile([P, NSLICE], dt.float16, name=f"osb{i}")
                   for i in range(NT)]
            cast_cnt = [0]

            def cast(dst, src, engine):
                # GPSIMD cannot read PSUM; only Scalar(ACT)/Vector can drain
                if engine == 0:
                    nc.scalar.copy(out=dst, in_=src)
                else:
                    nc.vector.tensor_copy(out=dst, in_=src)

            for j in range(NT):
                if j == 1:
                    finish_vt(v12, 0)
                    finish_vt(v12, 1)
                    finish_vt(v3, 0)
                ob = osb[j]
                for gi, (c0, gw) in enumerate(GROUPS):
                    mm = psm.tile([P, 1024], dt.float32, space="PSUM", tag="mm")
                    for lo in range(0, gw, 512):
                        cw = min(512, gw - lo)
                        nc.tensor.matmul(out=mm[:, lo:lo + cw],
                                         lhsT=vts[j][:],
                                         rhs=e0t[:, c0 + lo:c0 + lo + cw],
                                         start=True, stop=True)
                    eng = cast_cnt[0] % 2   # alternate scalar/vector
                    cast_cnt[0] += 1
                    cast(ob[:, c0:c0 + gw], mm[:, 0:gw], eng)
                    if gi % 2 == 1:
                        oc, ow = GROUPS[gi - 1][0], GROUPS[gi - 1][1] + gw
                        nc.sync.dma_start(
                            OUT[j * P:(j + 1) * P, oc:oc + ow],
                            ob[:, oc:oc + ow])

    nc.compile()
    return nc


def _prep_inputs(inputs):
    x = np.asarray(inputs["x"])
    E0 = np.asarray(inputs["E0"], dtype=np.float32)
    E1 = np.asarray(inputs["E1"], dtype=np.float32)
    E2 = np.asarray(inputs["E2"], dtype=np.float32)
    E3 = np.asarray(inputs["E3"], dtype=np.float32)
    E4 = np.asarray(inputs["E4"], dtype=np.float32)
    E5 = np.asarray(inputs["E5"], dtype=np.float32)
    E6 = np.asarray(inputs["E6"], dtype=np.float32)
    rule_C = np.asarray(inputs["rule_C"], dtype=np.float32)
    rule_S = np.asarray(inputs["rule_S"], dtype=np.float32)
    has_rules = np.asarray(inputs["has_rules"])

    h, r, t = x[:, 0].astype(np.int64), x[:, 1].astype(np.int64), x[:, 3].astype(np.int64)
    tb = t // CYCLE
    H = RANK // 2

    L = E0[h]
    R = E1[r]
    RC = rule_C[r]
    CT = E4[t]
    TM = E2[t] + E5[tb]
    TE = E3[t] + E6[tb]
    hr = has_rules[r].astype(np.float32)[:, None]
    hsr = hr * rule_S[r][:, None]

    def hs(a):
        return a[:, :H], a[:, H:]

    L0, L1 = hs(L)
    R0, R1 = hs(R)
    RC0, RC1 = hs(RC)
    C0, C1 = hs(CT)

    tblex = np.concatenate([
        R0, R0, R1, -R1,          # RELX4
        RC0, RC1, -RC1, RC0,      # RCP
        C0, C0, C1, C1,           # CTdup
        TM, TE,
        L0, L1, -L1, -L0,         # E0GX
        np.repeat(hr, RANK, axis=1),
        hsr * R,
    ], axis=1).astype(np.float16)   # [B, TW]
    assert tblex.shape[1] == TW

    e0t = np.ascontiguousarray(E0.T.astype(np.float16))   # [128, 40000]

    tbl_by_bh = []
    for bh in range(BS):
        rows = tblex[bh * ROWS:(bh + 1) * ROWS]
        tbl_by_bh.append(np.ascontiguousarray(
            rows.reshape(NT, P, TW).transpose(1, 0, 2)))
    e0t_by_es = [np.ascontiguousarray(e0t[:, es * NSLICE:(es + 1) * NSLICE])
                 for es in range(ES)]

    ident = np.eye(P, dtype=np.float16)
    in_maps = []
    for c in range(NCORES):
        in_maps.append({
            "TBL": tbl_by_bh[c // ES],
            "E0T": e0t_by_es[c % ES],
            "IDN": ident,
        })
    return in_maps


def kernel(**inputs):
    from concourse.bass_utils import run_bass_kernel_spmd

    if "nc" not in _CACHE:
        _CACHE["nc"] = _build()
    nc = _CACHE["nc"]

    in_maps = _prep_inputs(inputs)
    res = run_bass_kernel_spmd(nc, in_maps, core_ids=list(range(NCORES)),
                               trace=TRACE)
    _CACHE["last_result"] = res
    out = np.empty((B, NENT), np.float32)
    for c in range(NCORES):
        bh, es = c // ES, c % ES
        out[bh * ROWS:(bh + 1) * ROWS,
            es * NSLICE:(es + 1) * NSLICE] = res.results[c]["OUT"]
    return out
